# revision 45
# baseline (speedup 1.0000x reference)
"""ConcatRelationModule Bass kernel for 8 trn2 NeuronCores.

Reference computation (per edge e in [0, 16383)):
    x      = concat(inputs[heads[e], 0, :], inputs[e + 1, 1, :])     # [512]
    h      = tanh(concat(x @ W_FOH, x @ W_FOM) + b1)                 # [1024]
    h2     = tanh(h @ W2 + b2)                                       # [256]
    out[e] = h2 @ W3 + b3                                            # [64]

Strategy: data-parallel over edges (2048 per core, last edge padded).
Features live on SBUF partitions, edges on the free dim; work runs in
four 512-edge groups.  The modifier half of x is contiguous rows, so
the host ships it pre-transposed in the exact SBUF image (one DMA
line per partition).  The head half is gathered on-chip with indirect
DMAs and flipped to feature-major with PE transposes -- except for the
first two groups: the gather pipeline (index load -> offset DMA -> SW
queue) has ~8us of latency after the fixed engine-boot preamble, and
the PE clock needs ~3us of gap-free execution to reach full speed, so
the host pre-gathers the first 1024 edges (6% of rows) to give the PE
a stall-free runway while the remaining gathers stream in.  Each DMA
queue pays a ~2.4us completion-semaphore latency on top of ~220GB/s
service, so the ramp loads are split into small pieces ordered so the
first matmul's two operands finish first on the earliest-starting
queue.  Each group's L3 chain is deferred behind the next group's L1
so the L2-activation latency never stalls the PE, and the last
group's L3/out chain is split in half to shorten the serial tail
before the end-of-kernel barrier.  Output is produced as [64, E] per
core and transposed back on host.
"""

import os

import numpy as np
import ml_dtypes

import concourse.bass as bass
import concourse.bacc as bacc
import concourse.mybir as mybir
import concourse.tile as tile
from concourse.bass import IndirectOffsetOnAxis
from concourse.bass_utils import run_bass_kernel_spmd
from concourse.masks import make_identity

N_TOKENS = 16384
LD = 256          # ldims
HID = 512
HID2 = 256
NREL = 64
NCORES = 8
E = N_TOKENS - 1  # 16383 real edges
EPC = N_TOKENS // NCORES  # 2048 edges per core (padded)
P = 128
SUBTILES = EPC // P       # 16 subtiles of 128 edges
G = 512                   # edges per group
NG = EPC // G             # 4 groups
NPRE = 2                  # leading groups with host-pre-gathered heads
PRE = NPRE * G            # 1024 pre-gathered edges

# matmul operand dtype ("bf16" or "f32")
RUN_DT = os.environ.get("KERNEL_DT", "bf16")

LAST_RESULTS = None
_CACHE = {}


def _build(dt_str):
    cdt = mybir.dt.bfloat16 if dt_str == "bf16" else mybir.dt.float32
    f32 = mybir.dt.float32

    nc = bacc.Bacc()
    fwd = nc.declare_dram_parameter("fwd", [N_TOKENS, LD], cdt, isOutput=False)
    # first PRE edges of x, feature-major, in SBUF image layout:
    # per partition, groups of (4 k-chunks x 512 edges) contiguous
    xpre = nc.declare_dram_parameter("xpre", [P, NPRE * 4 * G], cdt,
                                     isOutput=False)
    # modifier halves of the remaining groups, same layout
    bwdG = nc.declare_dram_parameter("bwdG", [P, (NG - NPRE) * 2 * G], cdt,
                                     isOutput=False)
    headsT = nc.declare_dram_parameter(
        "headsT", [P, SUBTILES], mybir.dt.int32, isOutput=False)
    w1 = nc.declare_dram_parameter("w1", [2 * LD, 2 * HID], cdt, isOutput=False)
    # w2/w3 host-packed in SBUF image layout (one DMA line per partition)
    w2 = nc.declare_dram_parameter("w2", [P, 8 * HID2], cdt, isOutput=False)
    w3 = nc.declare_dram_parameter("w3", [P, 2 * NREL], cdt, isOutput=False)
    b1 = nc.declare_dram_parameter("b1", [P, 8], f32, isOutput=False)
    b2 = nc.declare_dram_parameter("b2", [P, 2], f32, isOutput=False)
    b3 = nc.declare_dram_parameter("b3", [NREL, 1], f32, isOutput=False)
    outT = nc.declare_dram_parameter("outT", [NREL, EPC], f32, isOutput=True)

    Tanh = mybir.ActivationFunctionType.Tanh
    Identity = mybir.ActivationFunctionType.Identity

    with tile.TileContext(nc) as tc:
        with (
            tc.tile_pool(name="const", bufs=1) as const_pool,
            tc.tile_pool(name="xh", bufs=8) as xh_pool,
            tc.tile_pool(name="xg", bufs=6) as xg_pool,
            tc.tile_pool(name="xm", bufs=2) as xm_pool,
            tc.tile_pool(name="xT", bufs=2) as xT_pool,
            tc.tile_pool(name="h1", bufs=16) as h1_pool,
            tc.tile_pool(name="h2", bufs=8) as h2_pool,
            tc.tile_pool(name="outs", bufs=3) as out_pool,
            tc.tile_pool(name="ph", bufs=4, space="PSUM") as ph_pool,
            tc.tile_pool(name="pj", bufs=2, space="PSUM") as pj_pool,
            tc.tile_pool(name="pt", bufs=2, space="PSUM") as pt_pool,
        ):
            po_pool = pj_pool  # L3 psum shares the pj banks

            # identity first: the PE warm-up chain below needs it ~7.4us in
            ident = const_pool.tile([P, P], cdt)
            make_identity(nc, ident[:])

            # PE clock warm-up: the tensor clock ramps to full speed only
            # after a few us of sustained execution, and the first real
            # matmul can't start before ~12us (DMA queue + semaphore
            # latency).  A chain of short dummy matmuls bridges the wait
            # so real work starts at a higher clock; short ones keep the
            # bridge-length granularity fine so the chain ends close to
            # when the first operands land.
            scratch = pt_pool.tile([P, 4 * P], f32, tag="pt", name="warmup")
            for _ in range(40):
                nc.tensor.matmul(out=scratch[:, 0:P], lhsT=ident[:],
                                 rhs=ident[:], start=True, stop=True)

            # --- sync HWDGE queue (earliest to start servicing): the
            # pieces that gate the first matmuls, smallest-first so their
            # completion semaphores fire as early as possible ---
            # w1 chunk 0 is split in hc-halves (hc 0-3 / 4-7) so the first
            # matmul's weights complete as early as possible
            w1c0_sb = [const_pool.tile([P, HID], cdt, tag=f"w1_0_{h}",
                                       name=f"w1_0_{h}") for h in range(2)]
            w1_sb = [None] + [const_pool.tile([P, 2 * HID], cdt,
                                              tag=f"w1_{kc}", name=f"w1_{kc}")
                              for kc in (1, 2, 3)]
            nc.sync.dma_start(w1c0_sb[0][:], w1[0:P, 0:HID])
            # pre-gathered x of group 0, one small load per k-chunk: each
            # gates the next slab of L1 matmuls
            xg0 = [xg_pool.tile([P, G], cdt, tag=f"xg0_{kc}", name=f"xg0_{kc}")
                   for kc in range(4)]
            for kc in range(4):
                nc.sync.dma_start(xg0[kc][:], xpre[:, kc * G:(kc + 1) * G])
            nc.sync.dma_start(w1c0_sb[1][:], w1[0:P, HID:2 * HID])
            hT_sb = const_pool.tile([P, SUBTILES], mybir.dt.int32)
            nc.sync.dma_start(hT_sb[:], headsT[:])
            # modifier halves of the gather groups: one load on the gpsimd
            # SW queue (not needed until ~28us), emitted before the gathers
            # so its descriptor generation isn't blocked behind them
            xm23 = xm_pool.tile([P, 2, 2, G], cdt, tag="xm", name="xm23")
            nc.gpsimd.dma_start(
                xm23[:], bwdG[:].rearrange("p (g k d) -> p g k d", g=2, k=2))
            xm_tiles = [None, None, xm23[:, 0], xm23[:, 1]]
            # group 1 in two halves
            xg1 = [xg_pool.tile([P, 2, G], cdt, tag=f"xg1_{h}", name=f"xg1_{h}")
                   for h in range(2)]
            for h in range(2):
                nc.sync.dma_start(
                    xg1[h][:],
                    xpre[:, (4 + 2 * h) * G:(6 + 2 * h) * G]
                    .rearrange("p (k d) -> p k d", k=2))

            # gathers for the non-pre-gathered subtiles, serial on gpsimd
            xh_tiles = []
            for s in range(PRE // P, SUBTILES):
                xh = xh_pool.tile([P, LD], cdt, tag="xh", name=f"xh_{s}")
                nc.gpsimd.indirect_dma_start(
                    out=xh[:],
                    out_offset=None,
                    in_=fwd[:],
                    in_offset=IndirectOffsetOnAxis(ap=hT_sb[:, s:s + 1], axis=0),
                )
                xh_tiles.append(xh)

            # --- scalar HWDGE queue (starts ~2us later, behind the
            # activation-table load): remaining weights + biases.  Few,
            # large loads: every DMA issue here delays the activations
            # that share the scalar engine. ---
            b1_sb = const_pool.tile([P, 8], f32)
            nc.scalar.dma_start(b1_sb[:], b1[:])
            for kc in (1, 2, 3):
                nc.scalar.dma_start(w1_sb[kc][:], w1[kc * P:(kc + 1) * P, :])
            b2_sb = const_pool.tile([P, 2], f32)
            nc.scalar.dma_start(b2_sb[:], b2[:])
            w2_sb = const_pool.tile([P, 8, HID2], cdt)
            nc.scalar.dma_start(
                w2_sb[:], w2[:].rearrange("p (k j) -> p k j", k=8))

            w3_sb = const_pool.tile([P, 2, NREL], cdt)
            nc.sync.dma_start(
                w3_sb[:], w3[:].rearrange("p (k r) -> p k r", k=2))
            b3_sb = const_pool.tile([NREL, 1], f32)
            nc.sync.dma_start(b3_sb[:], b3[:])

            # gathered head halves: flip to feature-major on the PE.
            xT_tiles = [None] * NG

            def emit_transpose(gi):
                xT = xT_pool.tile([P, 2, G], cdt, tag="xT", name=f"xT_{gi}")
                for s in range(G // P):
                    src = xh_tiles[gi * (G // P) + s - PRE // P]
                    pt = pt_pool.tile([P, 2, P], cdt, tag="pt",
                                      name=f"pt_{gi}_{s}")
                    for di in range(2):
                        nc.tensor.transpose(
                            pt[:, di, :], src[:, di * P:(di + 1) * P], ident[:])
                    nc.vector.tensor_copy(
                        out=xT[:, :, s * P:(s + 1) * P], in_=pt[:])
                xT_tiles[gi] = xT

            def l1_rhs(gi):
                if gi == 0:
                    return [(kc, xg0[kc][:]) for kc in range(4)]
                if gi == 1:
                    return [(0, xg1[0][:, 0, :]), (1, xg1[0][:, 1, :]),
                            (2, xg1[1][:, 0, :]), (3, xg1[1][:, 1, :])]
                # modifier k-chunks first: available before gathers
                xT, xm = xT_tiles[gi], xm_tiles[gi]
                return [(2, xm[:, 0, :]), (3, xm[:, 1, :]),
                        (0, xT[:, 0, :]), (1, xT[:, 1, :])]

            def l3_chain(gi, halves):
                start = gi * G
                h2sh = h2_tiles[gi]
                for hi, (hoff, hsize) in enumerate(halves):
                    po = po_pool.tile([NREL, hsize], f32, tag="pj",
                                      name=f"po_{gi}_{hi}")
                    for kc in range(2):
                        nc.tensor.matmul(
                            out=po[:],
                            lhsT=w3_sb[:, kc, :],
                            rhs=h2sh[hi][kc][:],
                            start=(kc == 0),
                            stop=(kc == 1),
                        )
                    o = out_pool.tile([NREL, hsize], f32, tag="o",
                                      name=f"o_{gi}_{hi}")
                    nc.scalar.activation(
                        out=o[:], in_=po[:], func=Identity, bias=b3_sb[:, 0:1]
                    )
                    last = gi == NG - 1
                    eng = nc.scalar if last and hi == 1 else nc.sync
                    eng.dma_start(
                        outT[:, start + hoff:start + hoff + hsize], o[:])

            h2_tiles = [None] * NG

            for gi in range(NG):
                size = G
                # ---- layer 1: h = tanh(W1.T-chunks @ x + b1), 8 h-chunks ----
                korder = l1_rhs(gi)
                h1s = []
                for hc in range(8):
                    ph = ph_pool.tile([P, size], f32, tag="ph",
                                      name=f"ph_{gi}_{hc}")
                    for ki, (kc, rhs) in enumerate(korder):
                        if kc == 0:
                            lhsT = w1c0_sb[hc // 4][:, (hc % 4) * P:
                                                    (hc % 4 + 1) * P]
                        else:
                            lhsT = w1_sb[kc][:, hc * P:(hc + 1) * P]
                        nc.tensor.matmul(
                            out=ph[:],
                            lhsT=lhsT,
                            rhs=rhs,
                            start=(ki == 0),
                            stop=(ki == 3),
                        )
                    h1 = h1_pool.tile([P, size], cdt, tag="h1",
                                      name=f"h1_{gi}_{hc}")
                    nc.scalar.activation(
                        out=h1[:], in_=ph[:], func=Tanh,
                        bias=b1_sb[:, hc:hc + 1],
                    )
                    h1s.append(h1)

                # previous group's L3 chain goes here: its h2 activations
                # have finished during this group's L1, so the L3 matmuls
                # never wait on the scalar engine
                if gi > 0:
                    l3_chain(gi - 1, ((0, G),))

                # ---- layer 2: h2 = tanh(W2-chunks @ h + b2), 2 j-chunks ----
                # The last group runs L2 in edge-halves so the first
                # half's L3/out chain overlaps the second half's matmuls.
                last = gi == NG - 1
                halves = ((0, 384), (384, 128)) if last else ((0, size),)
                h2_tiles[gi] = []
                for hi, (hoff, hsize) in enumerate(halves):
                    h2s = []
                    for jc in range(2):
                        pj = pj_pool.tile([P, hsize], f32, tag="pj",
                                          name=f"pj_{gi}_{jc}_{hi}")
                        for kc in range(8):
                            nc.tensor.matmul(
                                out=pj[:],
                                lhsT=w2_sb[:, kc, jc * P:(jc + 1) * P],
                                rhs=h1s[kc][:, hoff:hoff + hsize],
                                start=(kc == 0),
                                stop=(kc == 7),
                            )
                        h2 = h2_pool.tile([P, hsize], cdt, tag="h2",
                                          name=f"h2_{gi}_{jc}_{hi}")
                        nc.scalar.activation(
                            out=h2[:], in_=pj[:],
                            func=Tanh, bias=b2_sb[:, jc:jc + 1],
                        )
                        h2s.append(h2)
                    h2_tiles[gi].append(h2s)

                # transpose the NEXT gather-group: its gathers have landed
                if gi + 1 >= NPRE and gi + 1 < NG:
                    emit_transpose(gi + 1)

                if last:
                    l3_chain(gi, halves)

    nc.finalize()
    return nc


def kernel(inputs, rhidLayerFOH, rhidLayerFOM, rcatBias, rhid2Layer, rhid2Bias,
           routLayer, routBias, heads):
    global LAST_RESULTS

    inputs = np.asarray(inputs, dtype=np.float32)
    heads = np.asarray(heads)

    if RUN_DT == "bf16":
        wdt = ml_dtypes.bfloat16
    else:
        wdt = np.float32

    fwd = np.ascontiguousarray(inputs[:, 0, :]).astype(wdt)      # [N, 256]
    fwd32 = inputs[:, 0, :]
    bwd_full = inputs[:, 1, :]                                   # [N, 256]
    # mods for edge e is e+1; pad edge 16383 with mod 16383 (garbage, dropped)
    mods_pad = np.concatenate([np.arange(1, N_TOKENS), [N_TOKENS - 1]]).astype(np.int64)
    heads_pad = np.concatenate([heads.astype(np.int64), [0]]).astype(np.int64)

    w1 = np.ascontiguousarray(
        np.concatenate([np.asarray(rhidLayerFOH), np.asarray(rhidLayerFOM)], axis=1)
    ).astype(wdt)                                                # [512, 1024]
    # w2/w3 packed per partition: w2G[p, kc*256+j] = w2[kc*128+p, j]
    w2 = np.ascontiguousarray(
        np.asarray(rhid2Layer).reshape(8, P, HID2)
        .transpose(1, 0, 2).reshape(P, 8 * HID2)).astype(wdt)    # [128, 2048]
    w3 = np.ascontiguousarray(
        np.asarray(routLayer).reshape(2, P, NREL)
        .transpose(1, 0, 2).reshape(P, 2 * NREL)).astype(wdt)    # [128, 128]
    b1 = np.ascontiguousarray(
        np.asarray(rcatBias, dtype=np.float32).reshape(8, P).T)    # [128, 8]
    b2 = np.ascontiguousarray(
        np.asarray(rhid2Bias, dtype=np.float32).reshape(2, P).T)   # [128, 2]
    b3 = np.ascontiguousarray(
        np.asarray(routBias, dtype=np.float32).reshape(1, NREL).T)  # [64, 1]

    in_maps = []
    for c in range(NCORES):
        sl = slice(c * EPC, (c + 1) * EPC)
        hds = heads_pad[sl]
        mds = mods_pad[sl]
        # pre-gathered x image for the first PRE edges:
        # [P, NPRE groups * (4 k-chunks * G edges)]
        blocks = []
        for gi in range(NPRE):
            esl = slice(gi * G, (gi + 1) * G)
            fg = fwd32[hds[esl]].T.reshape(2, P, G)    # head half, [kc,p,e]
            bg = bwd_full[mds[esl]].T.reshape(2, P, G)  # mod half
            blocks.append(np.concatenate([fg, bg], 0)
                          .transpose(1, 0, 2).reshape(P, 4 * G))
        xpre_c = np.ascontiguousarray(np.concatenate(blocks, 1)).astype(wdt)
        # modifier halves of the remaining groups, same per-partition layout
        blocks = []
        for gi in range(NPRE, NG):
            esl = slice(gi * G, (gi + 1) * G)
            bg = bwd_full[mds[esl]].T.reshape(2, P, G)
            blocks.append(bg.transpose(1, 0, 2).reshape(P, 2 * G))
        bwdG_c = np.ascontiguousarray(np.concatenate(blocks, 1)).astype(wdt)
        headsT_c = np.ascontiguousarray(
            hds.astype(np.int32).reshape(SUBTILES, P).T)          # [128, 16]
        in_maps.append({
            "fwd": fwd, "xpre": xpre_c, "bwdG": bwdG_c, "headsT": headsT_c,
            "w1": w1, "w2": w2, "w3": w3, "b1": b1, "b2": b2, "b3": b3,
        })

    if RUN_DT not in _CACHE:
        _CACHE[RUN_DT] = _build(RUN_DT)
    nc = _CACHE[RUN_DT]

    trace_dir = os.environ.get("KERNEL_TRACE_DIR") or None
    res = run_bass_kernel_spmd(nc, in_maps, list(range(NCORES)), tmpdir=trace_dir)
    LAST_RESULTS = res

    outT = np.concatenate([r["outT"] for r in res.results], axis=1)  # [64, 16384]
    return np.ascontiguousarray(outT.T[:E]).astype(np.float32)       # [16383, 64]


# revision 47
# speedup vs baseline: 1.0377x; 1.0377x over previous
"""ConcatRelationModule Bass kernel for 8 trn2 NeuronCores.

Reference computation (per edge e in [0, 16383)):
    x      = concat(inputs[heads[e], 0, :], inputs[e + 1, 1, :])     # [512]
    h      = tanh(concat(x @ W_FOH, x @ W_FOM) + b1)                 # [1024]
    h2     = tanh(h @ W2 + b2)                                       # [256]
    out[e] = h2 @ W3 + b3                                            # [64]

Strategy: data-parallel over edges (2048 per core, last edge padded).
Features live on SBUF partitions, edges on the free dim; work runs in
four 512-edge groups.  The modifier half of x is contiguous rows, so
the host ships it pre-transposed in the exact SBUF image (one DMA
line per partition).  The head half is gathered on-chip with indirect
DMAs and flipped to feature-major with PE transposes -- except for the
first two groups: the gather pipeline (index load -> offset DMA -> SW
queue) has ~8us of latency after the fixed engine-boot preamble, and
the PE clock needs ~3us of gap-free execution to reach full speed, so
the host pre-gathers the first 1024 edges (6% of rows) to give the PE
a stall-free runway while the remaining gathers stream in.  Each DMA
queue pays a ~2.4us completion-semaphore latency on top of ~220GB/s
service, so the ramp loads are split into small pieces ordered so the
first matmul's two operands finish first on the earliest-starting
queue.  Each group's L3 chain is deferred behind the next group's L1
so the L2-activation latency never stalls the PE, and the last
group's L3/out chain is split in half to shorten the serial tail
before the end-of-kernel barrier.  Output is produced as [64, E] per
core and transposed back on host.
"""

import os

import numpy as np
import ml_dtypes

import concourse.bass as bass
import concourse.bacc as bacc
import concourse.mybir as mybir
import concourse.tile as tile
from concourse.bass import IndirectOffsetOnAxis
from concourse.bass_utils import run_bass_kernel_spmd
from concourse.masks import make_identity

N_TOKENS = 16384
LD = 256          # ldims
HID = 512
HID2 = 256
NREL = 64
NCORES = 8
E = N_TOKENS - 1  # 16383 real edges
EPC = N_TOKENS // NCORES  # 2048 edges per core (padded)
P = 128
SUBTILES = EPC // P       # 16 subtiles of 128 edges
G = 512                   # edges per group
NG = EPC // G             # 4 groups
NPRE = 2                  # leading groups with host-pre-gathered heads
PRE = NPRE * G            # 1024 pre-gathered edges

# matmul operand dtype ("bf16" or "f32")
RUN_DT = os.environ.get("KERNEL_DT", "bf16")

LAST_RESULTS = None
_CACHE = {}


def _build(dt_str):
    cdt = mybir.dt.bfloat16 if dt_str == "bf16" else mybir.dt.float32
    f32 = mybir.dt.float32

    nc = bacc.Bacc()
    fwd = nc.declare_dram_parameter("fwd", [N_TOKENS, LD], cdt, isOutput=False)
    # first PRE edges of x, feature-major, in SBUF image layout:
    # per partition, groups of (4 k-chunks x 512 edges) contiguous
    xpre = nc.declare_dram_parameter("xpre", [P, NPRE * 4 * G], cdt,
                                     isOutput=False)
    # modifier halves of the remaining groups, same layout
    bwdG = nc.declare_dram_parameter("bwdG", [P, (NG - NPRE) * 2 * G], cdt,
                                     isOutput=False)
    headsT = nc.declare_dram_parameter(
        "headsT", [P, SUBTILES], mybir.dt.int32, isOutput=False)
    w1 = nc.declare_dram_parameter("w1", [2 * LD, 2 * HID], cdt, isOutput=False)
    # w2/w3 host-packed in SBUF image layout (one DMA line per partition)
    w2 = nc.declare_dram_parameter("w2", [P, 8 * HID2], cdt, isOutput=False)
    w3 = nc.declare_dram_parameter("w3", [P, 2 * NREL], cdt, isOutput=False)
    b1 = nc.declare_dram_parameter("b1", [P, 8], f32, isOutput=False)
    b2 = nc.declare_dram_parameter("b2", [P, 2], f32, isOutput=False)
    b3 = nc.declare_dram_parameter("b3", [NREL, 1], f32, isOutput=False)
    outT = nc.declare_dram_parameter("outT", [NREL, EPC], f32, isOutput=True)

    Tanh = mybir.ActivationFunctionType.Tanh
    Identity = mybir.ActivationFunctionType.Identity

    with tile.TileContext(nc) as tc:
        with (
            tc.tile_pool(name="const", bufs=1) as const_pool,
            tc.tile_pool(name="xh", bufs=8) as xh_pool,
            tc.tile_pool(name="xg", bufs=6) as xg_pool,
            tc.tile_pool(name="xm", bufs=2) as xm_pool,
            tc.tile_pool(name="xT", bufs=2) as xT_pool,
            tc.tile_pool(name="h1", bufs=16) as h1_pool,
            tc.tile_pool(name="h2", bufs=8) as h2_pool,
            tc.tile_pool(name="outs", bufs=3) as out_pool,
            tc.tile_pool(name="ph", bufs=4, space="PSUM") as ph_pool,
            tc.tile_pool(name="pj", bufs=2, space="PSUM") as pj_pool,
            tc.tile_pool(name="pt", bufs=2, space="PSUM") as pt_pool,
        ):
            po_pool = pj_pool  # L3 psum shares the pj banks

            # identity first: the PE warm-up chain below needs it ~7.4us in
            ident = const_pool.tile([P, P], cdt)
            make_identity(nc, ident[:])

            # PE clock warm-up: the tensor clock ramps to full speed only
            # after a few us of sustained execution, and the first real
            # matmul can't start before ~12us (DMA queue + semaphore
            # latency).  A chain of short dummy matmuls bridges the wait
            # so real work starts at a higher clock; short ones keep the
            # bridge-length granularity fine so the chain ends close to
            # when the first operands land.
            scratch = pt_pool.tile([P, 4 * P], f32, tag="pt", name="warmup")
            for _ in range(40):
                nc.tensor.matmul(out=scratch[:, 0:P], lhsT=ident[:],
                                 rhs=ident[:], start=True, stop=True)

            # --- sync HWDGE queue (earliest to start servicing): the
            # pieces that gate the first matmuls, smallest-first so their
            # completion semaphores fire as early as possible ---
            # w1 chunk 0 is split in hc-halves (hc 0-3 / 4-7) so the first
            # matmul's weights complete as early as possible
            w1c0_sb = [const_pool.tile([P, HID], cdt, tag=f"w1_0_{h}",
                                       name=f"w1_0_{h}") for h in range(2)]
            w1_sb = [None] + [const_pool.tile([P, 2 * HID], cdt,
                                              tag=f"w1_{kc}", name=f"w1_{kc}")
                              for kc in (1, 2, 3)]
            nc.sync.dma_start(w1c0_sb[0][:], w1[0:P, 0:HID])
            # pre-gathered x of group 0, one small load per k-chunk: each
            # gates the next slab of L1 matmuls
            xg0 = [xg_pool.tile([P, G], cdt, tag=f"xg0_{kc}", name=f"xg0_{kc}")
                   for kc in range(4)]
            for kc in range(4):
                nc.sync.dma_start(xg0[kc][:], xpre[:, kc * G:(kc + 1) * G])
            nc.sync.dma_start(w1c0_sb[1][:], w1[0:P, HID:2 * HID])
            hT_sb = const_pool.tile([P, SUBTILES], mybir.dt.int32)
            nc.sync.dma_start(hT_sb[:], headsT[:])
            # group 1 in two halves
            xg1 = [xg_pool.tile([P, 2, G], cdt, tag=f"xg1_{h}", name=f"xg1_{h}")
                   for h in range(2)]
            for h in range(2):
                nc.sync.dma_start(
                    xg1[h][:],
                    xpre[:, (4 + 2 * h) * G:(6 + 2 * h) * G]
                    .rearrange("p (k d) -> p k d", k=2))

            # gathers for the non-pre-gathered subtiles, serial on gpsimd
            xh_tiles = []
            for s in range(PRE // P, SUBTILES):
                xh = xh_pool.tile([P, LD], cdt, tag="xh", name=f"xh_{s}")
                nc.gpsimd.indirect_dma_start(
                    out=xh[:],
                    out_offset=None,
                    in_=fwd[:],
                    in_offset=IndirectOffsetOnAxis(ap=hT_sb[:, s:s + 1], axis=0),
                )
                xh_tiles.append(xh)

            # --- scalar HWDGE queue (starts ~2us later, behind the
            # activation-table load): remaining weights + biases.  Few,
            # large loads: every DMA issue here delays the activations
            # that share the scalar engine. ---
            b1_sb = const_pool.tile([P, 8], f32)
            nc.scalar.dma_start(b1_sb[:], b1[:])
            for kc in (1, 2, 3):
                nc.scalar.dma_start(w1_sb[kc][:], w1[kc * P:(kc + 1) * P, :])
            b2_sb = const_pool.tile([P, 2], f32)
            nc.scalar.dma_start(b2_sb[:], b2[:])
            w2_sb = const_pool.tile([P, 8, HID2], cdt)
            nc.scalar.dma_start(
                w2_sb[:], w2[:].rearrange("p (k j) -> p k j", k=8))

            # modifier halves of the gather groups: one load (4KB lines)
            xm23 = xm_pool.tile([P, 2, 2, G], cdt, tag="xm", name="xm23")
            nc.sync.dma_start(
                xm23[:], bwdG[:].rearrange("p (g k d) -> p g k d", g=2, k=2))
            xm_tiles = [None, None, xm23[:, 0], xm23[:, 1]]

            w3_sb = const_pool.tile([P, 2, NREL], cdt)
            nc.sync.dma_start(
                w3_sb[:], w3[:].rearrange("p (k r) -> p k r", k=2))
            b3_sb = const_pool.tile([NREL, 1], f32)
            nc.sync.dma_start(b3_sb[:], b3[:])

            # gathered head halves: flip to feature-major on the PE.
            xT_tiles = [None] * NG

            def emit_transpose(gi):
                xT = xT_pool.tile([P, 2, G], cdt, tag="xT", name=f"xT_{gi}")
                for s in range(G // P):
                    src = xh_tiles[gi * (G // P) + s - PRE // P]
                    pt = pt_pool.tile([P, 2, P], cdt, tag="pt",
                                      name=f"pt_{gi}_{s}")
                    for di in range(2):
                        nc.tensor.transpose(
                            pt[:, di, :], src[:, di * P:(di + 1) * P], ident[:])
                    nc.vector.tensor_copy(
                        out=xT[:, :, s * P:(s + 1) * P], in_=pt[:])
                xT_tiles[gi] = xT

            def l1_rhs(gi):
                if gi == 0:
                    return [(kc, xg0[kc][:]) for kc in range(4)]
                if gi == 1:
                    return [(0, xg1[0][:, 0, :]), (1, xg1[0][:, 1, :]),
                            (2, xg1[1][:, 0, :]), (3, xg1[1][:, 1, :])]
                # modifier k-chunks first: available before gathers
                xT, xm = xT_tiles[gi], xm_tiles[gi]
                return [(2, xm[:, 0, :]), (3, xm[:, 1, :]),
                        (0, xT[:, 0, :]), (1, xT[:, 1, :])]

            def l3_chain(gi, halves):
                start = gi * G
                h2sh = h2_tiles[gi]
                for hi, (hoff, hsize) in enumerate(halves):
                    po = po_pool.tile([NREL, hsize], f32, tag="pj",
                                      name=f"po_{gi}_{hi}")
                    for kc in range(2):
                        nc.tensor.matmul(
                            out=po[:],
                            lhsT=w3_sb[:, kc, :],
                            rhs=h2sh[hi][kc][:],
                            start=(kc == 0),
                            stop=(kc == 1),
                        )
                    o = out_pool.tile([NREL, hsize], f32, tag="o",
                                      name=f"o_{gi}_{hi}")
                    nc.scalar.activation(
                        out=o[:], in_=po[:], func=Identity, bias=b3_sb[:, 0:1]
                    )
                    last = gi == NG - 1
                    eng = nc.scalar if last and hi == 1 else nc.sync
                    eng.dma_start(
                        outT[:, start + hoff:start + hoff + hsize], o[:])

            h2_tiles = [None] * NG

            for gi in range(NG):
                size = G
                # ---- layer 1: h = tanh(W1.T-chunks @ x + b1), 8 h-chunks ----
                korder = l1_rhs(gi)
                h1s = []
                for hc in range(8):
                    ph = ph_pool.tile([P, size], f32, tag="ph",
                                      name=f"ph_{gi}_{hc}")
                    for ki, (kc, rhs) in enumerate(korder):
                        if kc == 0:
                            lhsT = w1c0_sb[hc // 4][:, (hc % 4) * P:
                                                    (hc % 4 + 1) * P]
                        else:
                            lhsT = w1_sb[kc][:, hc * P:(hc + 1) * P]
                        nc.tensor.matmul(
                            out=ph[:],
                            lhsT=lhsT,
                            rhs=rhs,
                            start=(ki == 0),
                            stop=(ki == 3),
                        )
                    h1 = h1_pool.tile([P, size], cdt, tag="h1",
                                      name=f"h1_{gi}_{hc}")
                    nc.scalar.activation(
                        out=h1[:], in_=ph[:], func=Tanh,
                        bias=b1_sb[:, hc:hc + 1],
                    )
                    h1s.append(h1)

                # previous group's L3 chain goes here: its h2 activations
                # have finished during this group's L1, so the L3 matmuls
                # never wait on the scalar engine
                if gi > 0:
                    l3_chain(gi - 1, ((0, G),))

                # ---- layer 2: h2 = tanh(W2-chunks @ h + b2), 2 j-chunks ----
                # The last group runs L2 in edge-halves so the first
                # half's L3/out chain overlaps the second half's matmuls.
                last = gi == NG - 1
                halves = ((0, 384), (384, 128)) if last else ((0, size),)
                h2_tiles[gi] = []
                for hi, (hoff, hsize) in enumerate(halves):
                    h2s = []
                    for jc in range(2):
                        pj = pj_pool.tile([P, hsize], f32, tag="pj",
                                          name=f"pj_{gi}_{jc}_{hi}")
                        for kc in range(8):
                            nc.tensor.matmul(
                                out=pj[:],
                                lhsT=w2_sb[:, kc, jc * P:(jc + 1) * P],
                                rhs=h1s[kc][:, hoff:hoff + hsize],
                                start=(kc == 0),
                                stop=(kc == 7),
                            )
                        h2 = h2_pool.tile([P, hsize], cdt, tag="h2",
                                          name=f"h2_{gi}_{jc}_{hi}")
                        nc.scalar.activation(
                            out=h2[:], in_=pj[:],
                            func=Tanh, bias=b2_sb[:, jc:jc + 1],
                        )
                        h2s.append(h2)
                    h2_tiles[gi].append(h2s)

                # transpose the NEXT gather-group: its gathers have landed
                if gi + 1 >= NPRE and gi + 1 < NG:
                    emit_transpose(gi + 1)

                if last:
                    l3_chain(gi, halves)

    nc.finalize()
    return nc


def kernel(inputs, rhidLayerFOH, rhidLayerFOM, rcatBias, rhid2Layer, rhid2Bias,
           routLayer, routBias, heads):
    global LAST_RESULTS

    inputs = np.asarray(inputs, dtype=np.float32)
    heads = np.asarray(heads)

    if RUN_DT == "bf16":
        wdt = ml_dtypes.bfloat16
    else:
        wdt = np.float32

    fwd = np.ascontiguousarray(inputs[:, 0, :]).astype(wdt)      # [N, 256]
    fwd32 = inputs[:, 0, :]
    bwd_full = inputs[:, 1, :]                                   # [N, 256]
    # mods for edge e is e+1; pad edge 16383 with mod 16383 (garbage, dropped)
    mods_pad = np.concatenate([np.arange(1, N_TOKENS), [N_TOKENS - 1]]).astype(np.int64)
    heads_pad = np.concatenate([heads.astype(np.int64), [0]]).astype(np.int64)

    w1 = np.ascontiguousarray(
        np.concatenate([np.asarray(rhidLayerFOH), np.asarray(rhidLayerFOM)], axis=1)
    ).astype(wdt)                                                # [512, 1024]
    # w2/w3 packed per partition: w2G[p, kc*256+j] = w2[kc*128+p, j]
    w2 = np.ascontiguousarray(
        np.asarray(rhid2Layer).reshape(8, P, HID2)
        .transpose(1, 0, 2).reshape(P, 8 * HID2)).astype(wdt)    # [128, 2048]
    w3 = np.ascontiguousarray(
        np.asarray(routLayer).reshape(2, P, NREL)
        .transpose(1, 0, 2).reshape(P, 2 * NREL)).astype(wdt)    # [128, 128]
    b1 = np.ascontiguousarray(
        np.asarray(rcatBias, dtype=np.float32).reshape(8, P).T)    # [128, 8]
    b2 = np.ascontiguousarray(
        np.asarray(rhid2Bias, dtype=np.float32).reshape(2, P).T)   # [128, 2]
    b3 = np.ascontiguousarray(
        np.asarray(routBias, dtype=np.float32).reshape(1, NREL).T)  # [64, 1]

    in_maps = []
    for c in range(NCORES):
        sl = slice(c * EPC, (c + 1) * EPC)
        hds = heads_pad[sl]
        mds = mods_pad[sl]
        # pre-gathered x image for the first PRE edges:
        # [P, NPRE groups * (4 k-chunks * G edges)]
        blocks = []
        for gi in range(NPRE):
            esl = slice(gi * G, (gi + 1) * G)
            fg = fwd32[hds[esl]].T.reshape(2, P, G)    # head half, [kc,p,e]
            bg = bwd_full[mds[esl]].T.reshape(2, P, G)  # mod half
            blocks.append(np.concatenate([fg, bg], 0)
                          .transpose(1, 0, 2).reshape(P, 4 * G))
        xpre_c = np.ascontiguousarray(np.concatenate(blocks, 1)).astype(wdt)
        # modifier halves of the remaining groups, same per-partition layout
        blocks = []
        for gi in range(NPRE, NG):
            esl = slice(gi * G, (gi + 1) * G)
            bg = bwd_full[mds[esl]].T.reshape(2, P, G)
            blocks.append(bg.transpose(1, 0, 2).reshape(P, 2 * G))
        bwdG_c = np.ascontiguousarray(np.concatenate(blocks, 1)).astype(wdt)
        headsT_c = np.ascontiguousarray(
            hds.astype(np.int32).reshape(SUBTILES, P).T)          # [128, 16]
        in_maps.append({
            "fwd": fwd, "xpre": xpre_c, "bwdG": bwdG_c, "headsT": headsT_c,
            "w1": w1, "w2": w2, "w3": w3, "b1": b1, "b2": b2, "b3": b3,
        })

    if RUN_DT not in _CACHE:
        _CACHE[RUN_DT] = _build(RUN_DT)
    nc = _CACHE[RUN_DT]

    trace_dir = os.environ.get("KERNEL_TRACE_DIR") or None
    res = run_bass_kernel_spmd(nc, in_maps, list(range(NCORES)), tmpdir=trace_dir)
    LAST_RESULTS = res

    outT = np.concatenate([r["outT"] for r in res.results], axis=1)  # [64, 16384]
    return np.ascontiguousarray(outT.T[:E]).astype(np.float32)       # [16383, 64]
